# revision 11
# baseline (speedup 1.0000x reference)
"""Baichuan transformer layer on 8 Trainium2 NeuronCores (Megatron TP-8).

v3 dataflow (per core, SPMD). Changes vs v2, driven by the NTFF profile
(v2: 2.27ms HW, TensorE 72% busy, MLP floors ~270us late, 256-wide MLP
pieces, 60us exposed final RS):

  - MLP runs as four 256-wide pieces, one per AllGather chunk, each
    floored to start right as its gather lands, so gate matmuls
    backfill the attention region's TensorE idle (dense TensorE traffic
    also keeps the PE at its warm clock: cold matmuls measure ~2x);
  - the per-chunk h2 = hs + attention-RS shard (already formed in bf16
    as the AllGather source) is kept in SBUF, so the tail after the
    last down-RS is just load + one add + store;
  - each piece's down-proj feeds its own ReduceScatter + assembly, so
    the final collectives pipeline;
  - one PSUM pool for the whole kernel (7x2KB "ps" rotation + 2x1KB
    "tp" for transposes/l-sums);
  - everything else (transposed activations, bf16 matmuls, RMSNorm
    scales folded into PSUM evacuation, 4x256 attention chunks with
    RS->residual->AG chains) as in v2.
"""

import math

import numpy as np
import ml_dtypes

import concourse.bass as bass
import concourse.mybir as mybir
import concourse.tile as tile
from concourse import bacc
from concourse.bass_utils import run_bass_kernel_spmd
from concourse.masks import make_identity
from concourse.alu_op_type import AluOpType

F32 = mybir.dt.float32
F32R = mybir.dt.float32r
BF16 = mybir.dt.bfloat16
NPBF16 = ml_dtypes.bfloat16

N_CORES = 8
S = 1024          # tokens
H = 5120          # hidden
HK = H // 128     # 40 hidden k-tiles
NH = 40           # heads total
NH_SH = NH // N_CORES   # 5 heads per core
HD = 128          # head dim
F = NH_SH * HD    # 640 attn features per core
FK = F // 128     # 5 shard k-tiles
INTER = 13696
ISH = INTER // N_CORES  # 1712 inter features per core
IK = (ISH + 127) // 128  # 14 inter k-tiles (last = 48 rows)
EPS = 1e-6

NCH = 4                # attention / collective chunks
CW = S // NCH          # 256 tokens per collective chunk
MW = 512               # MLP piece width (2 pieces of 512)
ST = S // 128          # 8 token 128-tiles

QKV_GRP = 3            # qkv m-chunks per psum group (x2 s-halves = 6 bufs)
OP_GRP = 4             # o_proj out-tiles per psum group
GU_GRP = 7             # gate/up out-tiles per psum group
DN_GRP = 7             # down out-tiles per psum group

FLOORS = [0.26, 0.31, 0.36, 0.41]  # scheduler floors (virtual ms) per piece


def build_nc():
    nc = bacc.Bacc("TRN2", target_bir_lowering=False, debug=False,
                   num_devices=N_CORES)

    # ---- I/O ----
    hT = nc.dram_tensor("hT", [H, S], BF16, kind="ExternalInput")
    hs = nc.dram_tensor("hs", [F, S], BF16, kind="ExternalInput")
    maskT = nc.dram_tensor("maskT", [NH_SH, 128, ST, S], BF16,
                           kind="ExternalInput")
    wp = nc.dram_tensor("wp", [H, 3 * F], BF16, kind="ExternalInput")
    wo = nc.dram_tensor("wo", [F, H], BF16, kind="ExternalInput")
    wgu = nc.dram_tensor("wgu", [H, 2 * ISH], BF16, kind="ExternalInput")
    wd = nc.dram_tensor("wd", [ISH, H], BF16, kind="ExternalInput")
    out = nc.dram_tensor("out", [F, S], F32, kind="ExternalOutput")

    # ---- internal DRAM (collective bounce buffers) ----
    ra_in = [nc.dram_tensor(f"ra_in{c}", [H, CW], BF16) for c in range(NCH)]
    ra_out = [nc.dram_tensor(f"ra_out{c}", [F, CW], BF16)
              for c in range(NCH)]
    ag_in = [nc.dram_tensor(f"ag_in{c}", [F, CW], BF16) for c in range(NCH)]
    ag_out = [nc.dram_tensor(f"ag_out{c}", [H, CW], BF16, addr_space="Shared")
              for c in range(NCH)]
    rm_in = [nc.dram_tensor(f"rm_in{m}", [H, CW], BF16) for m in range(NCH)]
    rm_out = [nc.dram_tensor(f"rm_out{m}", [F, CW], BF16)
              for m in range(NCH)]
    RG = [list(range(N_CORES))]

    with tile.TileContext(nc) as tc:
        with tc.tile_pool(name="const", bufs=1) as constp:
            ones_b = constp.tile([128, 1], BF16, tag="ones_b")
            nc.any.memset(ones_b[:], 1.0)
            onesr_f32 = constp.tile([1, 128], F32, tag="onesr_f32")
            nc.any.memset(onesr_f32[:], 1.0)
            onesr_f = constp.tile([1, 128], F32R, tag="onesr_f")
            nc.vector.tensor_copy(onesr_f[:], onesr_f32[:])
            ident_b = constp.tile([128, 128], BF16, tag="ident_b")
            make_identity(nc, ident_b)

            # single PSUM pool for the whole kernel: 7x2KB + 2x1KB = 16KB
            aps = tc.alloc_tile_pool(name="aps", bufs=7, space="PSUM")

            # persistent SBUF pools
            agp = tc.alloc_tile_pool(name="agp", bufs=2)
            o1pool = tc.alloc_tile_pool(name="o1pool", bufs=1)
            hsp = tc.alloc_tile_pool(name="hsp", bufs=1)
            qkp = tc.alloc_tile_pool(name="qk_pool", bufs=1)
            qT = [qkp.tile([128, S], BF16, tag=f"qT{h}", name=f"qT{h}")
                  for h in range(NH_SH)]
            kT = [qkp.tile([128, S], BF16, tag=f"kT{h}", name=f"kT{h}")
                  for h in range(NH_SH)]
            vn = [qkp.tile([128, F], BF16, tag=f"vn{s}", name=f"vn{s}")
                  for s in range(ST)]
            aop = tc.alloc_tile_pool(name="ao_pool", bufs=2)

            # X = bf16(hT) resident through QKV
            xpool = tc.alloc_tile_pool(name="xpool", bufs=1)
            X = [xpool.tile([128, S], BF16, tag=f"x{k}", name=f"x{k}")
                 for k in range(HK)]
            scp = tc.alloc_tile_pool(name="scp", bufs=1)
            sc1b = scp.tile([128, S], F32, tag="sc1b")

            # ============ phase A: cast X + rms1 stats ============
            p0pool = tc.alloc_tile_pool(name="p0", bufs=3)
            ss_ps = [aps.tile([1, 512], F32, tag="ps", name=f"ss_ps{i}")
                     for i in range(2)]
            for k in range(HK):
                eng = nc.sync if k % 2 == 0 else nc.scalar
                eng.dma_start(out=X[k][:], in_=hT[k * 128:(k + 1) * 128, :])
                sqb = p0pool.tile([128, S], BF16, tag="sqb")
                if k % 2 == 0:
                    nc.gpsimd.tensor_mul(sqb[:], X[k][:], X[k][:])
                else:
                    nc.vector.tensor_mul(sqb[:], X[k][:], X[k][:])
                for half in range(2):
                    nc.tensor.matmul(
                        ss_ps[half][:], ones_b[:],
                        sqb[:, half * 512:(half + 1) * 512],
                        start=(k == 0), stop=(k == HK - 1))
            s1row = scp.tile([1, S], F32, tag="s1row")
            for half in range(2):
                hsl = slice(half * 512, (half + 1) * 512)
                nc.vector.tensor_scalar(
                    s1row[:, hsl], ss_ps[half][:], 1.0 / H, EPS,
                    AluOpType.mult, AluOpType.add)
            s1r2 = scp.tile([1, S], F32, tag="s1r2")
            nc.vector.reciprocal(s1r2[:], s1row[:])
            s1r3 = scp.tile([1, S], F32R, tag="s1r3")
            with nc.allow_low_precision(reason="fp32r scale row"):
                nc.scalar.sqrt(s1r3[:], s1r2[:])      # rsqrt(mean+eps)
            for half in range(2):
                hsl = slice(half * 512, (half + 1) * 512)
                bps = aps.tile([128, 512], F32, tag="ps", name="bps1")
                nc.tensor.matmul(bps[:], onesr_f[:], s1r3[:, hsl],
                                 start=True, stop=True)
                nc.scalar.copy(sc1b[:, hsl], bps[:])
            p0pool.release()

            # ============ phase B: QKV (scale folded into evac) ============
            qkvstr = tc.alloc_tile_pool(name="qkvstr", bufs=3)
            n_mch = 3 * NH_SH  # 15 col chunks of the w_pack shard
            for g0 in range(0, n_mch, QKV_GRP):
                gsz = min(QKV_GRP, n_mch - g0)
                pst = [[aps.tile([128, 512], F32, tag="ps",
                                 name=f"qkvps{mi}_{half}")
                        for half in range(2)] for mi in range(gsz)]
                for k in range(HK):
                    wsl = qkvstr.tile([128, QKV_GRP * 128], BF16, tag="wp_sl")
                    nc.sync.dma_start(
                        out=wsl[:, :gsz * 128],
                        in_=wp[k * 128:(k + 1) * 128,
                               g0 * 128:(g0 + gsz) * 128])
                    for mi in range(gsz):
                        for half in range(2):
                            nc.tensor.matmul(
                                pst[mi][half][:],
                                wsl[:, mi * 128:(mi + 1) * 128],
                                X[k][:, half * 512:(half + 1) * 512],
                                start=(k == 0), stop=(k == HK - 1))
                for mi in range(gsz):
                    m = g0 + mi
                    for half in range(2):
                        hsl = slice(half * 512, (half + 1) * 512)
                        if m < NH_SH:
                            nc.vector.tensor_mul(qT[m][:, hsl],
                                                 pst[mi][half][:],
                                                 sc1b[:, hsl])
                        elif m < 2 * NH_SH:
                            nc.vector.tensor_mul(kT[m - NH_SH][:, hsl],
                                                 pst[mi][half][:],
                                                 sc1b[:, hsl])
                        else:
                            h = m - 2 * NH_SH
                            vt = qkvstr.tile([128, 512], BF16, tag="vT_ev",
                                             bufs=2)
                            nc.vector.tensor_mul(vt[:], pst[mi][half][:],
                                                 sc1b[:, hsl])
                            for sb in range(4):
                                s_tile = half * 4 + sb
                                tps = aps.tile([128, 128], BF16, tag="tp",
                                               bufs=1, name="tps")
                                nc.tensor.transpose(
                                    tps[:], vt[:, sb * 128:(sb + 1) * 128],
                                    ident_b[:])
                                nc.scalar.copy(
                                    vn[s_tile][:, h * 128:(h + 1) * 128],
                                    tps[:])
            qkvstr.release()
            scp.release()
            xpool.release()

            # ============ phase C/D pools (reuse X's freed region) ========
            mlstr = tc.alloc_tile_pool(name="mlstr", bufs=3)
            mlpc = tc.alloc_tile_pool(name="mlpc", bufs=2)
            gup = tc.alloc_tile_pool(name="gup", bufs=1)
            attnstr = tc.alloc_tile_pool(name="attnstr", bufs=2)
            expp = tc.alloc_tile_pool(name="exp_pool", bufs=11)

            hs3 = hs.rearrange("(k p) i -> p k i", p=128)
            hsres = hsp.tile([128, FK * S], BF16, tag="hsres")
            nc.sync.dma_start(
                out=hsres[:].rearrange("p (k i) -> p k i", k=FK), in_=hs3)

            def hssl(ci):
                return hsres[:].rearrange(
                    "p (k i) -> p k i", k=FK)[:, :, ci]

            h2p = [o1pool.tile([128, FK * CW], BF16, tag=f"h2p{c}",
                               name=f"h2p{c}") for c in range(NCH)]

            # ============ phase C: attention chunks + RS/AG chains ========
            def emit_scores(h, c):
                ci = slice(c * CW, (c + 1) * CW)
                mk8 = attnstr.tile([128, ST * CW], BF16, tag="mk8", bufs=2)
                nc.sync.dma_start(
                    out=mk8[:].rearrange("p (j i) -> p j i", j=ST),
                    in_=maskT[h, :, :, ci])
                ets = []
                for jp in range(ST // 2):
                    # two key-tiles share one psum bank -> [128, 512] chain
                    sps = aps.tile([128, 2 * CW], F32, tag="ps", name="sps")
                    for w in range(2):
                        j = 2 * jp + w
                        nc.tensor.matmul(
                            sps[:, w * CW:(w + 1) * CW],
                            kT[h][:, j * 128:(j + 1) * 128],
                            qT[h][:, ci], start=True, stop=True)
                    scf = attnstr.tile([128, 2 * CW], BF16, tag="sc_f",
                                       bufs=2)
                    nc.vector.tensor_add(
                        scf[:], sps[:],
                        mk8[:, jp * 2 * CW:(jp + 1) * 2 * CW])
                    et = expp.tile([128, 2 * CW], BF16, tag="expT", name="et")
                    nc.scalar.activation(
                        et[:], scf[:], mybir.ActivationFunctionType.Exp)
                    ets.append(et)
                return ets

            def emit_post(h, ets, c, aoT):
                l_ps = aps.tile([1, CW], F32, tag="tp", bufs=1, name="l_ps")
                for jp in range(ST // 2):
                    for w in range(2):
                        nc.tensor.matmul(
                            l_ps[:], ones_b[:],
                            ets[jp][:, w * CW:(w + 1) * CW],
                            start=(jp == 0 and w == 0),
                            stop=(jp == ST // 2 - 1 and w == 1))
                inv = attnstr.tile([1, CW], F32R, tag="inv_l", bufs=2)
                with nc.allow_low_precision(reason="f32r inv"):
                    nc.vector.reciprocal(inv[:], l_ps[:])
                ibp = aps.tile([128, CW], F32, tag="ps", name="ibp")
                nc.tensor.matmul(ibp[:], onesr_f[:], inv[:],
                                 start=True, stop=True)
                ibs = attnstr.tile([128, CW], F32, tag="ib_s", bufs=2)
                nc.scalar.copy(ibs[:], ibp[:])
                avp = aps.tile([128, CW], F32, tag="ps", name="avp")
                for jp in range(ST // 2):
                    for w in range(2):
                        j = 2 * jp + w
                        nc.tensor.matmul(
                            avp[:], vn[j][:, h * 128:(h + 1) * 128],
                            ets[jp][:, w * CW:(w + 1) * CW],
                            start=(j == 0), stop=(j == ST - 1))
                nc.vector.tensor_mul(aoT[h][:], avp[:], ibs[:])

            for c in range(NCH):
                ci = slice(c * CW, (c + 1) * CW)
                aoT = [aop.tile([128, CW], BF16, tag=f"aoT{h}",
                                name=f"aoT{h}_{c}") for h in range(NH_SH)]
                # two-deep stagger: scores run two heads ahead of post
                pend = [emit_scores(0, c), emit_scores(1, c)]
                for h in range(2, NH_SH):
                    cur = emit_scores(h, c)
                    emit_post(h - 2, pend[0], c, aoT)
                    pend = [pend[1], cur]
                emit_post(NH_SH - 2, pend[0], c, aoT)
                emit_post(NH_SH - 1, pend[1], c, aoT)

                # ---- o_proj partials ----
                for g0 in range(0, HK, OP_GRP):
                    gsz = min(OP_GRP, HK - g0)
                    pst = [aps.tile([128, CW], F32, tag="ps",
                                    name=f"ops{mi}") for mi in range(gsz)]
                    for f in range(NH_SH):
                        wosl = attnstr.tile([128, OP_GRP * 128], BF16,
                                            tag="wo_sl", bufs=3)
                        eng = nc.sync if f % 2 == 0 else nc.scalar
                        eng.dma_start(
                            out=wosl[:, :gsz * 128],
                            in_=wo[f * 128:(f + 1) * 128,
                                   g0 * 128:(g0 + gsz) * 128])
                        for mi in range(gsz):
                            nc.tensor.matmul(
                                pst[mi][:],
                                wosl[:, mi * 128:(mi + 1) * 128],
                                aoT[f][:],
                                start=(f == 0), stop=(f == NH_SH - 1))
                    ob = attnstr.tile([128, OP_GRP * CW], BF16,
                                      tag="o_ev", bufs=2)
                    for mi in range(gsz):
                        nc.vector.tensor_copy(ob[:, mi * CW:(mi + 1) * CW],
                                              pst[mi][:])
                    nc.sync.dma_start(
                        out=ra_in[c].rearrange(
                            "(g p) i -> p g i", p=128)[:, g0:g0 + gsz, :],
                        in_=ob[:, :gsz * CW].rearrange(
                            "p (g i) -> p g i", g=gsz))
                nc.gpsimd.collective_compute(
                    "ReduceScatter", mybir.AluOpType.add,
                    ins=[ra_in[c][:, :].opt()], outs=[ra_out[c][:, :].opt()],
                    replica_groups=RG)

                # ---- residual add + AllGather, all on the gpsimd queue ----
                arb = agp.tile([128, FK * CW], BF16, tag="arb", bufs=1)
                nc.gpsimd.dma_start(
                    out=arb[:].rearrange("p (k i) -> p k i", k=FK),
                    in_=ra_out[c].rearrange("(k p) i -> p k i", p=128))
                nc.gpsimd.tensor_add(
                    h2p[c][:].rearrange("p (k i) -> p k i", k=FK), hssl(ci),
                    arb[:].rearrange("p (k i) -> p k i", k=FK))
                nc.gpsimd.dma_start(
                    out=ag_in[c].rearrange("(k p) i -> p k i", p=128),
                    in_=h2p[c][:].rearrange("p (k i) -> p k i", k=FK))
                nc.gpsimd.collective_compute(
                    "AllGather", mybir.AluOpType.bypass,
                    ins=[ag_in[c][:, :].opt()], outs=[ag_out[c][:, :].opt()],
                    replica_groups=RG)

            # ============ phase D: four 256-wide MLP pieces ============
            # One piece per AllGather chunk, floored to start right as its
            # gather lands, so MLP matmuls backfill the attention region's
            # TensorE idle (which also keeps the PE at its warm clock).
            def emit_assembly(c):
                # out = (hs + attn shard) [kept from phase C] + mlp shard
                cci = slice(c * CW, (c + 1) * CW)
                rmb = agp.tile([128, FK * CW], BF16, tag="rmb", bufs=1)
                nc.gpsimd.dma_start(
                    out=rmb[:].rearrange("p (k i) -> p k i", k=FK),
                    in_=rm_out[c].rearrange("(k p) i -> p k i", p=128))
                o1 = agp.tile([128, FK * CW], F32, tag="o1", bufs=1)
                nc.gpsimd.tensor_add(o1[:], h2p[c][:], rmb[:])
                nc.gpsimd.dma_start(
                    out=out.rearrange(
                        "(k p) i -> p k i", p=128)[:, :, cci],
                    in_=o1[:].rearrange("p (k i) -> p k i", k=FK))

            def mlp_piece(c):
                W = CW
                yHc = gup.tile([128, HK * W], BF16, tag="yHc", bufs=2,
                               name=f"yHc{c}")
                # split the gather-in load across two HWDGE queues to
                # halve its exposed latency at the piece boundary
                src3 = ag_out[c].rearrange("(k p) i -> p k i", p=128)
                dst3 = yHc[:].rearrange("p (k i) -> p k i", k=HK)
                nc.sync.dma_start(out=dst3[:, :HK // 2, :],
                                  in_=src3[:, :HK // 2, :])
                nc.scalar.dma_start(out=dst3[:, HK // 2:, :],
                                    in_=src3[:, HK // 2:, :])
                # ---- ln2 stats from gathered h2 ----
                ss2 = aps.tile([1, W], F32, tag="ps", name="ss2")
                for k in range(HK):
                    ysl = yHc[:, k * W:(k + 1) * W]
                    sq2 = mlstr.tile([128, W], BF16, tag="sq2", bufs=2)
                    nc.vector.tensor_mul(sq2[:], ysl, ysl)
                    nc.tensor.matmul(ss2[:], ones_b[:], sq2[:],
                                     start=(k == 0), stop=(k == HK - 1))
                s2a = mlpc.tile([1, W], F32, tag="s2a", bufs=2)
                nc.vector.tensor_scalar(s2a[:], ss2[:], 1.0 / H, EPS,
                                        AluOpType.mult, AluOpType.add)
                s2b = mlpc.tile([1, W], F32, tag="s2b", bufs=2)
                nc.vector.reciprocal(s2b[:], s2a[:])
                s2c = mlpc.tile([1, W], F32R, tag="s2c", bufs=2)
                with nc.allow_low_precision(reason="fp32r scale row"):
                    nc.scalar.sqrt(s2c[:], s2b[:])
                bps = aps.tile([128, W], F32, tag="ps", name="bps2")
                nc.tensor.matmul(bps[:], onesr_f[:], s2c[:],
                                 start=True, stop=True)
                sc2b = mlpc.tile([128, W], F32, tag="sc2b", bufs=2)
                nc.scalar.copy(sc2b[:], bps[:])

                # ---- gate then up (scale folded into evac) ----
                gsH = gup.tile([128, IK * W], BF16, tag="gsH", bufs=2,
                               name=f"gsH{c}")
                gu = [gup.tile([128, W], BF16, tag=f"gu_{m}", bufs=2,
                               name=f"gu{m}_{c}") for m in range(IK)]
                for wgt_i in range(2):
                    for g0 in range(0, IK, GU_GRP):
                        gsz = min(GU_GRP, IK - g0)
                        pst = [aps.tile([128, W], F32, tag="ps",
                                        name=f"gups{mi}")
                               for mi in range(gsz)]
                        c0w = wgt_i * ISH + g0 * 128
                        c1w = min(c0w + gsz * 128, wgt_i * ISH + ISH)
                        for k in range(HK):
                            wsl = mlstr.tile([128, GU_GRP * 128], BF16,
                                             tag="gu_sl")
                            eng = nc.sync if k % 2 == 0 else nc.scalar
                            eng.dma_start(
                                out=wsl[:, :c1w - c0w],
                                in_=wgu[k * 128:(k + 1) * 128, c0w:c1w])
                            for mi in range(gsz):
                                mw = min(128, ISH - (g0 + mi) * 128)
                                nc.tensor.matmul(
                                    pst[mi][:mw, :],
                                    wsl[:, mi * 128:mi * 128 + mw],
                                    yHc[:, k * W:(k + 1) * W],
                                    start=(k == 0), stop=(k == HK - 1))
                        for mi in range(gsz):
                            m = g0 + mi
                            mw = min(128, ISH - m * 128)
                            if wgt_i == 0:
                                gsc = mlstr.tile([128, W], BF16,
                                                 tag="gsc", bufs=2)
                                nc.vector.tensor_mul(gsc[:mw, :],
                                                     pst[mi][:mw, :],
                                                     sc2b[:mw, :])
                                nc.scalar.activation(
                                    gsH[:mw, m * W:(m + 1) * W],
                                    gsc[:mw, :],
                                    mybir.ActivationFunctionType.Silu)
                            else:
                                usc = mlstr.tile([128, W], BF16,
                                                 tag="usc", bufs=2)
                                nc.vector.tensor_mul(usc[:mw, :],
                                                     pst[mi][:mw, :],
                                                     sc2b[:mw, :])
                                nc.vector.tensor_mul(
                                    gu[m][:mw, :], usc[:mw, :],
                                    gsH[:mw, m * W:(m + 1) * W])

                # ---- down partials -> rm_in -> RS -> assembly ----
                for g0 in range(0, HK, DN_GRP):
                    gsz = min(DN_GRP, HK - g0)
                    pst = [aps.tile([128, W], F32, tag="ps",
                                    name=f"dps{mi}")
                           for mi in range(gsz)]
                    for k in range(IK):
                        kw = min(128, ISH - k * 128)
                        wsl = mlstr.tile([128, DN_GRP * 128], BF16,
                                         tag="dn_sl")
                        eng = nc.sync if k % 2 == 0 else nc.scalar
                        eng.dma_start(
                            out=wsl[:kw, :gsz * 128],
                            in_=wd[k * 128:k * 128 + kw,
                                   g0 * 128:(g0 + gsz) * 128])
                        for mi in range(gsz):
                            nc.tensor.matmul(
                                pst[mi][:],
                                wsl[:kw, mi * 128:(mi + 1) * 128],
                                gu[k][:kw, :],
                                start=(k == 0), stop=(k == IK - 1))
                    db = mlstr.tile([128, DN_GRP * CW], BF16,
                                    tag="d_ev", bufs=2)
                    for mi in range(gsz):
                        if mi % 2 == 0:
                            nc.vector.tensor_copy(
                                db[:, mi * CW:(mi + 1) * CW], pst[mi][:])
                        else:
                            nc.scalar.copy(
                                db[:, mi * CW:(mi + 1) * CW], pst[mi][:])
                    nc.sync.dma_start(
                        out=rm_in[c].rearrange(
                            "(g p) i -> p g i", p=128)[:, g0:g0 + gsz, :],
                        in_=db[:, :gsz * CW].rearrange(
                            "p (g i) -> p g i", g=gsz))
                nc.gpsimd.collective_compute(
                    "ReduceScatter", mybir.AluOpType.add,
                    ins=[rm_in[c][:, :].opt()],
                    outs=[rm_out[c][:, :].opt()],
                    replica_groups=RG)
                emit_assembly(c)

            for c in range(NCH):
                with tc.tile_wait_until(FLOORS[c]):
                    mlp_piece(c)

            expp.release()
            attnstr.release()
            gup.release()
            mlpc.release()
            mlstr.release()
            aop.release()
            qkp.release()
            hsp.release()
            o1pool.release()
            agp.release()
            aps.release()

    nc.compile()
    return nc


_NC_CACHE = None


def _get_nc():
    global _NC_CACHE
    if _NC_CACHE is None:
        _NC_CACHE = build_nc()
    return _NC_CACHE


def prepare_in_maps(hidden_states, attention_mask, w_pack, o_proj, gate_proj,
                    up_proj, down_proj, ln1_w, ln2_w):
    hidden_states = np.asarray(hidden_states, dtype=np.float32)
    attention_mask = np.asarray(attention_mask, dtype=np.float32)
    w_pack = np.asarray(w_pack, dtype=np.float32)
    o_proj = np.asarray(o_proj, dtype=np.float32)
    gate_proj = np.asarray(gate_proj, dtype=np.float32)
    up_proj = np.asarray(up_proj, dtype=np.float32)
    down_proj = np.asarray(down_proj, dtype=np.float32)
    ln1_w = np.asarray(ln1_w, dtype=np.float32)
    ln2_w = np.asarray(ln2_w, dtype=np.float32)

    hT = np.ascontiguousarray(hidden_states.reshape(S, H).T)  # [H, S] f32
    hT_bf = hT.astype(NPBF16)
    # fold ln1 into w_pack rows; fold 1/sqrt(HD) into the q columns
    wpf = (ln1_w[:, None] * w_pack).reshape(H, 3, NH, HD).copy()
    wpf[:, 0] *= 1.0 / math.sqrt(HD)
    wgf = (ln2_w[:, None] * gate_proj).astype(NPBF16)
    wuf = (ln2_w[:, None] * up_proj).astype(NPBF16)
    wdf = down_proj.astype(NPBF16)
    # mask[h, j_key, i_query] tiled for one-DMA loads:
    # [NH, 128, ST, S] with maskT[h, p, jt, i] = mask[h, jt*128+p, i]
    mask = np.ascontiguousarray(
        attention_mask.reshape(NH, S, S).transpose(0, 2, 1)
        .reshape(NH, ST, 128, S).transpose(0, 2, 1, 3)).astype(NPBF16)

    in_maps = []
    for c in range(N_CORES):
        hsl = slice(c * NH_SH, (c + 1) * NH_SH)
        wp_sh = np.ascontiguousarray(
            wpf[:, :, hsl, :].reshape(H, 3 * F)).astype(NPBF16)
        maskT_sh = np.ascontiguousarray(mask[hsl])
        wo_sh = np.ascontiguousarray(
            o_proj[c * F:(c + 1) * F, :]).astype(NPBF16)
        wgu_sh = np.ascontiguousarray(
            np.concatenate([wgf[:, c * ISH:(c + 1) * ISH],
                            wuf[:, c * ISH:(c + 1) * ISH]], axis=1))
        wd_sh = np.ascontiguousarray(wdf[c * ISH:(c + 1) * ISH, :])
        hs_sh = np.ascontiguousarray(hT[c * F:(c + 1) * F, :]).astype(NPBF16)
        in_maps.append({
            "hT": hT_bf, "hs": hs_sh, "maskT": maskT_sh, "wp": wp_sh,
            "wo": wo_sh, "wgu": wgu_sh, "wd": wd_sh,
        })
    return in_maps


def postprocess(results):
    outT = np.empty((H, S), dtype=np.float32)
    for c in range(N_CORES):
        outT[c * F:(c + 1) * F, :] = results[c]["out"]
    return np.ascontiguousarray(outT.T).reshape(1, S, H)


def kernel(**inputs):
    in_maps = prepare_in_maps(**inputs)
    nc = _get_nc()
    res = run_bass_kernel_spmd(nc, in_maps, list(range(N_CORES)))
    return postprocess(res.results)
